# revision 8
# baseline (speedup 1.0000x reference)
"""Trainium2 Bass kernel: nn_AttentionLayer (T=2048, B=2, H=16, N_in=1024, d=64).

Sharding: head-parallel across 8 NeuronCores. Each core gets the full x plus a
128-row slice (2 heads) of Wk/Wq/Wv and biases, computes attention for its 2
heads x 2 batches, and writes out[:, :, c*128:(c+1)*128]. The host concatenates
the 8 shards along the feature axis. No cross-core collectives.

Per-core math (matching the reference):
  K^T, Q^T, V^T = W_slice @ x_b^T + bias          (out^T layout, [128, T])
  S^T[k, i]     = sum_n Q^T[n,k] * K^T[n,i]        (k = softmax/query axis)
  E             = exp(S^T / 32)                    (bf16, safe range: |S/32| < ~1.5)
  out'[i, 0:64] = sum_k E[k,i] * V[k, :]           (A@V numerator)
  out'[i, 64]   = sum_k E[k,i]                     (softmax denominator via a
                                                    ones-column appended to V)
  out[i, :]     = out'[i, 0:64] / out'[i, 64]

x^T is produced by a f32->bf16 cast DMA (SWDGE) into a DRAM bounce followed by
DMA-xbar transposes (HWDGE); W^T and V are produced with PE matmul-by-identity
transposes. All matmuls run in bf16 (fp32 matmul is 2x slower on the PE).
"""

import numpy as np

T = 2048
B = 2
NIN = 1024
NQK = 64
NCORES = 8
H_PER_CORE = 2
GD = H_PER_CORE * NQK  # 128: per-core projection width (2 heads x 64)

NT = NIN // 128  # 8  n-tiles (contraction tiles for projections)
TT = T // 128    # 16 t-tiles
ICH = 2          # i-chunks per (b, h)
IC_LEN = T // ICH
IT = IC_LEN // 128  # 8 i-tiles per chunk
JT = TT          # 16 k-tiles (softmax axis)

_CACHE = {}


def _build():
    import concourse.bass as bass
    import concourse.tile as tile
    from concourse import bacc, mybir
    from concourse.masks import make_identity

    f32 = mybir.dt.float32
    bf16 = mybir.dt.bfloat16
    AF = mybir.ActivationFunctionType

    nc = bacc.Bacc("TRN2", target_bir_lowering=False, debug=False,
                   num_devices=NCORES)

    x = nc.dram_tensor("x", [T, B, NIN], f32, kind="ExternalInput").ap()
    w_in = {
        "k": nc.dram_tensor("wk", [GD, NIN], f32, kind="ExternalInput").ap(),
        "q": nc.dram_tensor("wq", [GD, NIN], f32, kind="ExternalInput").ap(),
        "v": nc.dram_tensor("wv", [GD, NIN], f32, kind="ExternalInput").ap(),
    }
    b_in = {
        "k": nc.dram_tensor("bk", [GD], f32, kind="ExternalInput").ap(),
        "q": nc.dram_tensor("bq", [GD], f32, kind="ExternalInput").ap(),
        "v": nc.dram_tensor("bv", [GD], f32, kind="ExternalInput").ap(),
    }
    out = nc.dram_tensor("out", [T, B, GD], f32, kind="ExternalOutput").ap()

    with tile.TileContext(nc) as tc:
        with (
            tc.tile_pool(name="const", bufs=1) as const_pool,
            tc.tile_pool(name="wstage", bufs=2) as wstage,
            tc.tile_pool(name="wt", bufs=1) as wt_pool,
            tc.tile_pool(name="xt", bufs=2) as xt_pool,
            tc.tile_pool(name="pt", bufs=1) as pt_pool,
            tc.tile_pool(name="vp", bufs=1) as vp_pool,
            tc.tile_pool(name="es", bufs=18) as es_pool,
            tc.tile_pool(name="of", bufs=3) as of_pool,
            tc.tile_pool(name="sm", bufs=4) as sm_pool,
            tc.tile_pool(name="dram", bufs=1, space="DRAM") as dram_pool,
            tc.tile_pool(name="ps_s", bufs=2, space="PSUM") as ps_s,
            tc.tile_pool(name="ps_av", bufs=2, space="PSUM") as ps_av,
        ):
            # --- constants -------------------------------------------------
            ident = const_pool.tile([128, 128], f32, name="ident")
            make_identity(nc, ident)
            identb = const_pool.tile([128, 128], bf16, name="identb")
            nc.vector.tensor_copy(out=identb[:], in_=ident[:])

            bias_t = {}
            for p in ("k", "q", "v"):
                bt = const_pool.tile([128, 1], f32, name=f"bias_{p}")
                nc.sync.dma_start(out=bt[:], in_=b_in[p].rearrange("(p o) -> p o", o=1))
                bias_t[p] = bt

            # --- weights: W [128, 1024] -> W^T tiles [128(n), nt, 128(g)] bf16
            wt = {}
            for p in ("k", "q", "v"):
                wnat = wstage.tile([128, NIN], f32, name=f"wnat_{p}", tag="wnat")
                nc.sync.dma_start(out=wnat[:], in_=w_in[p])
                wps = ps_av.tile([128, 2, 512], f32, name=f"wps_{p}", tag="av")
                for nt in range(NT):
                    nc.tensor.matmul(
                        wps[:, nt // 4, (nt % 4) * 128:(nt % 4) * 128 + 128],
                        lhsT=wnat[:, nt * 128:(nt + 1) * 128],
                        rhs=ident[:],
                        start=True, stop=True,
                    )
                w_t = wt_pool.tile([128, NT, 128], bf16, name=f"wt_{p}", tag=f"wt_{p}")
                nc.vector.tensor_copy(
                    out=w_t[:],
                    in_=wps.rearrange("p a (c m) -> p (a c) m", m=128),
                )
                wt[p] = w_t

            # --- per-batch: x^T, projections, V' --------------------------
            pt = {}   # pt[(p, b)]: [128(g), T] bf16   (g = 2 heads x 64)
            vp = {}   # vp[(h, b)]: [128(t), JT, 65] bf16 (V plus ones column)

            def prep_batch(b):
                xbf = dram_pool.tile([T, NIN], bf16, name=f"xbf_{b}", tag=f"xbf_{b}")
                xT = xt_pool.tile([128, NT, T], bf16, name=f"xT_{b}", tag="xT")
                # chunked cast + xbar transpose so downstream work starts early
                for cc in range(4):
                    rows = slice(cc * (T // 4), (cc + 1) * (T // 4))
                    nc.gpsimd.dma_start(out=xbf[rows, :], in_=x[rows, b, :])
                    for nt in range(NT):
                        nc.sync.dma_start(
                            out=xT[:, nt, cc * (T // 4):(cc + 1) * (T // 4)],
                            in_=xbf[rows, nt * 128:(nt + 1) * 128],
                            transpose=True,
                        )

                for p in ("k", "q", "v"):
                    ptile = pt_pool.tile([128, T], bf16, name=f"pt_{p}_{b}",
                                         tag=f"pt_{p}_{b}")
                    for ic in range(ICH):
                        pps = ps_s.tile([128, IC_LEN], f32, name=f"pps_{p}_{b}_{ic}",
                                        tag="s")
                        for s in range(2):
                            for nt in range(NT):
                                nc.tensor.matmul(
                                    pps[:, s * 512:(s + 1) * 512],
                                    lhsT=wt[p][:, nt, :],
                                    rhs=xT[:, nt,
                                           ic * IC_LEN + s * 512:
                                           ic * IC_LEN + (s + 1) * 512],
                                    start=(nt == 0), stop=(nt == NT - 1),
                                )
                        nc.vector.tensor_scalar_add(
                            out=ptile[:, ic * IC_LEN:(ic + 1) * IC_LEN],
                            in0=pps[:],
                            scalar1=bias_t[p][:],
                        )
                    pt[(p, b)] = ptile

                # V natural layout + ones column: transpose V^T with the PE
                for h in range(H_PER_CORE):
                    v_t = vp_pool.tile([128, JT, 65], bf16, name=f"vp_{h}_{b}",
                                       tag=f"vp_{h}_{b}")
                    nc.vector.memset(v_t[:, :, 64:65], 1.0)
                    vp[(h, b)] = v_t
                for grp in range(2):  # noqa: B007
                    vps = ps_av.tile([128, 2, 512], f32, name=f"vps_{b}_{grp}",
                                     tag="av")
                    for j in range(8):
                        tt = grp * 8 + j
                        nc.tensor.matmul(
                            vps[:, j // 4, (j % 4) * 128:(j % 4) * 128 + 128],
                            lhsT=pt[("v", b)][:, tt * 128:(tt + 1) * 128],
                            rhs=identb[:],
                            start=True, stop=True,
                        )
                    for s in range(2):
                        blk = vps[:, s, :].rearrange("p (c m) -> p c m", m=128)
                        for h in range(H_PER_CORE):
                            nc.vector.tensor_copy(
                                out=vp[(h, b)][:, grp * 8 + s * 4:
                                               grp * 8 + s * 4 + 4, 0:64],
                                in_=blk[:, :, h * 64:h * 64 + 64],
                            )

            # --- attention -------------------------------------------------
            out_v = out.rearrange("(ic it p) b (h n) -> ic b h p it n",
                                  it=IT, p=128, h=H_PER_CORE)

            def attn_batch(b):
                for h in range(H_PER_CORE):
                    qv = pt[("q", b)]
                    kv = pt[("k", b)]
                    for ich in range(ICH):
                        # scores + exp for all 16 k-tiles of this i-chunk
                        esl = []
                        for jt in range(JT):
                            sps = ps_s.tile([128, IC_LEN], f32,
                                            name=f"sps_{b}_{h}_{ich}_{jt}", tag="s")
                            for s in range(2):
                                nc.tensor.matmul(
                                    sps[:, s * 512:(s + 1) * 512],
                                    lhsT=qv[h * 64:(h + 1) * 64,
                                            jt * 128:(jt + 1) * 128],
                                    rhs=kv[h * 64:(h + 1) * 64,
                                           ich * IC_LEN + s * 512:
                                           ich * IC_LEN + (s + 1) * 512],
                                    start=True, stop=True,
                                )
                            es = es_pool.tile([128, IC_LEN], bf16,
                                              name=f"es_{b}_{h}_{ich}_{jt}", tag="es")
                            nc.scalar.activation(out=es[:], in_=sps[:], func=AF.Exp,
                                                 scale=1.0 / 32.0)
                            esl.append(es)
                        # A @ [V, 1]: one 1-bank accumulator per i-tile so each
                        # PSUM zero region holds exactly one accumulation group
                        linv = sm_pool.tile([128, 8, 1], f32,
                                            name=f"linv_{b}_{h}_{ich}", tag="linv")
                        outf = of_pool.tile([128, IT, 64], f32,
                                            name=f"outf_{b}_{h}_{ich}", tag="of")
                        for it in range(IT):
                            av = ps_av.tile([128, 65], f32,
                                            name=f"av_{b}_{h}_{ich}_{it}", tag="av")
                            for jt in range(JT):
                                nc.tensor.matmul(
                                    av[:],
                                    lhsT=esl[jt][:, it * 128:(it + 1) * 128],
                                    rhs=vp[(h, b)][:, jt, :],
                                    start=(jt == 0), stop=(jt == JT - 1),
                                )
                            lv = linv[:, it:it + 1, :]
                            nc.vector.reciprocal(out=lv, in_=av[:, 64:65])
                            rep = bass.AP(tensor=lv.tensor, offset=lv.offset,
                                          ap=[lv.ap[0], [0, 64]])
                            nc.vector.tensor_mul(
                                out=outf[:, it, :],
                                in0=av[:, 0:64],
                                in1=rep,
                            )
                        nc.sync.dma_start(out=out_v[ich, b, h], in_=outf[:])

            # batch 1's prep DMA/PE work overlaps batch 0's attention
            for b in range(B):
                prep_batch(b)
                attn_batch(b)
    nc.compile()  # bacc passes: regalloc, DCE, act-table loads, ...
    return nc


def _get_nc():
    if "nc" not in _CACHE:
        _CACHE["nc"] = _build()
    return _CACHE["nc"]


def run(inputs, trace=False, trace_kwargs=None):
    """Run on 8 NeuronCores. Returns (full_output, BassKernelResults)."""
    from concourse.bass_utils import run_bass_kernel_spmd

    nc = _get_nc()
    x = np.ascontiguousarray(np.asarray(inputs["x"], dtype=np.float32))
    in_maps = []
    for c in range(NCORES):
        sl = slice(c * GD, (c + 1) * GD)
        in_maps.append({
            "x": x,
            "wk": np.ascontiguousarray(np.asarray(inputs["Wk"], np.float32)[sl]),
            "wq": np.ascontiguousarray(np.asarray(inputs["Wq"], np.float32)[sl]),
            "wv": np.ascontiguousarray(np.asarray(inputs["Wv"], np.float32)[sl]),
            "bk": np.ascontiguousarray(np.asarray(inputs["bk"], np.float32)[sl]),
            "bq": np.ascontiguousarray(np.asarray(inputs["bq"], np.float32)[sl]),
            "bv": np.ascontiguousarray(np.asarray(inputs["bv"], np.float32)[sl]),
        })
    res = run_bass_kernel_spmd(nc, in_maps, core_ids=list(range(NCORES)),
                               trace=trace, **(trace_kwargs or {}))
    outs = [np.asarray(res.results[c]["out"]) for c in range(NCORES)]
    full = np.concatenate(outs, axis=2).astype(np.float32)
    return full, res


def kernel(x, mask, Wk, bk, Wq, bq, Wv, bv):
    """Full (unsharded) inputs -> full (T, B, H*N_V) float32 output.

    mask is all-True for this problem (spec fill: ones) and is ignored.
    """
    full, _ = run(dict(x=x, mask=mask, Wk=Wk, bk=bk, Wq=Wq, bq=bq, Wv=Wv, bv=bv))
    return full


# revision 10
# speedup vs baseline: 1.0786x; 1.0786x over previous
"""Trainium2 Bass kernel: nn_AttentionLayer (T=2048, B=2, H=16, N_in=1024, d=64).

Sharding: head-parallel across 8 NeuronCores. Each core gets the full x plus a
128-row slice (2 heads) of Wk/Wq/Wv and biases, computes attention for its 2
heads x 2 batches, and writes out[:, :, c*128:(c+1)*128]. The host concatenates
the 8 shards along the feature axis. No cross-core collectives.

Per-core math (matching the reference):
  K^T, Q^T, V^T = W_slice @ x_b^T + bias          (out^T layout, [128, T])
  S^T[k, i]     = sum_n Q^T[n,k] * K^T[n,i]        (k = softmax/query axis)
  E             = exp(S^T / 32)                    (bf16, safe range: |S/32| < ~1.5)
  out'[i, 0:64] = sum_k E[k,i] * V[k, :]           (A@V numerator)
  out'[i, 64]   = sum_k E[k,i]                     (softmax denominator via a
                                                    ones-column appended to V)
  out[i, :]     = out'[i, 0:64] / out'[i, 64]

x^T is produced by a f32->bf16 cast DMA (SWDGE) into a DRAM bounce followed by
DMA-xbar transposes (HWDGE); W^T and V are produced with PE matmul-by-identity
transposes. All matmuls run in bf16 (fp32 matmul is 2x slower on the PE).
"""

import numpy as np

T = 2048
B = 2
NIN = 1024
NQK = 64
NCORES = 8
H_PER_CORE = 2
GD = H_PER_CORE * NQK  # 128: per-core projection width (2 heads x 64)

NT = NIN // 128  # 8  n-tiles (contraction tiles for projections)
TT = T // 128    # 16 t-tiles
ICH = 2          # i-chunks per (b, h)
IC_LEN = T // ICH
IT = IC_LEN // 128  # 8 i-tiles per chunk
JT = TT          # 16 k-tiles (softmax axis)

_CACHE = {}


def _build():
    import concourse.bass as bass
    import concourse.tile as tile
    from concourse import bacc, mybir
    from concourse.masks import make_identity

    f32 = mybir.dt.float32
    bf16 = mybir.dt.bfloat16
    AF = mybir.ActivationFunctionType

    nc = bacc.Bacc("TRN2", target_bir_lowering=False, debug=False,
                   num_devices=NCORES)

    x = nc.dram_tensor("x", [T, B, NIN], f32, kind="ExternalInput").ap()
    w_in = {
        "k": nc.dram_tensor("wk", [GD, NIN], f32, kind="ExternalInput").ap(),
        "q": nc.dram_tensor("wq", [GD, NIN], f32, kind="ExternalInput").ap(),
        "v": nc.dram_tensor("wv", [GD, NIN], f32, kind="ExternalInput").ap(),
    }
    b_in = {
        "k": nc.dram_tensor("bk", [GD], f32, kind="ExternalInput").ap(),
        "q": nc.dram_tensor("bq", [GD], f32, kind="ExternalInput").ap(),
        "v": nc.dram_tensor("bv", [GD], f32, kind="ExternalInput").ap(),
    }
    out = nc.dram_tensor("out", [T, B, GD], f32, kind="ExternalOutput").ap()

    with tile.TileContext(nc) as tc:
        with (
            tc.tile_pool(name="const", bufs=1) as const_pool,
            tc.tile_pool(name="wstage", bufs=2) as wstage,
            tc.tile_pool(name="wt", bufs=1) as wt_pool,
            tc.tile_pool(name="xt", bufs=2) as xt_pool,
            tc.tile_pool(name="pt", bufs=1) as pt_pool,
            tc.tile_pool(name="vp", bufs=1) as vp_pool,
            tc.tile_pool(name="es", bufs=18) as es_pool,
            tc.tile_pool(name="of", bufs=3) as of_pool,
            tc.tile_pool(name="sm", bufs=4) as sm_pool,
            tc.tile_pool(name="dram", bufs=1, space="DRAM") as dram_pool,
            tc.tile_pool(name="ps_s", bufs=2, space="PSUM") as ps_s,
            tc.tile_pool(name="ps_av", bufs=2, space="PSUM") as ps_av,
        ):
            # --- constants -------------------------------------------------
            ident = const_pool.tile([128, 128], f32, name="ident")
            make_identity(nc, ident)
            identb = const_pool.tile([128, 128], bf16, name="identb")
            nc.vector.tensor_copy(out=identb[:], in_=ident[:])

            bias_t = {}
            for p in ("k", "q", "v"):
                bt = const_pool.tile([128, 1], f32, name=f"bias_{p}")
                nc.sync.dma_start(out=bt[:], in_=b_in[p].rearrange("(p o) -> p o", o=1))
                bias_t[p] = bt

            # --- weights: W [128, 1024] -> W^T tiles [128(n), nt, 128(g)] bf16
            wt = {}
            for p in ("k", "q", "v"):
                wnat = wstage.tile([128, NIN], f32, name=f"wnat_{p}", tag="wnat")
                nc.sync.dma_start(out=wnat[:], in_=w_in[p])
                wps = ps_av.tile([128, 2, 512], f32, name=f"wps_{p}", tag="av")
                for nt in range(NT):
                    nc.tensor.matmul(
                        wps[:, nt // 4, (nt % 4) * 128:(nt % 4) * 128 + 128],
                        lhsT=wnat[:, nt * 128:(nt + 1) * 128],
                        rhs=ident[:],
                        start=True, stop=True,
                    )
                w_t = wt_pool.tile([128, NT, 128], bf16, name=f"wt_{p}", tag=f"wt_{p}")
                nc.vector.tensor_copy(
                    out=w_t[:],
                    in_=wps.rearrange("p a (c m) -> p (a c) m", m=128),
                )
                wt[p] = w_t

            # --- per-batch: x^T, projections, V' --------------------------
            pt = {}   # pt[(p, b)]: [128(g), T] bf16   (g = 2 heads x 64)
            vp = {}   # vp[(h, b)]: [128(t), JT, 65] bf16 (V plus ones column)

            def prep_batch(b):
                xbf = dram_pool.tile([T, NIN], bf16, name=f"xbf_{b}", tag=f"xbf_{b}")
                nc.gpsimd.dma_start(out=xbf[:], in_=x[:, b, :])  # f32->bf16 cast
                xT = xt_pool.tile([128, NT, T], bf16, name=f"xT_{b}", tag="xT")
                for nt in range(NT):
                    nc.sync.dma_start(
                        out=xT[:, nt, :],
                        in_=xbf[:, nt * 128:(nt + 1) * 128],
                        transpose=True,
                    )

                for p in ("k", "q", "v"):
                    ptile = pt_pool.tile([128, T], bf16, name=f"pt_{p}_{b}",
                                         tag=f"pt_{p}_{b}")
                    for ic in range(ICH):
                        pps = ps_s.tile([128, IC_LEN], f32, name=f"pps_{p}_{b}_{ic}",
                                        tag="s")
                        for s in range(2):
                            for nt in range(NT):
                                nc.tensor.matmul(
                                    pps[:, s * 512:(s + 1) * 512],
                                    lhsT=wt[p][:, nt, :],
                                    rhs=xT[:, nt,
                                           ic * IC_LEN + s * 512:
                                           ic * IC_LEN + (s + 1) * 512],
                                    start=(nt == 0), stop=(nt == NT - 1),
                                )
                        nc.vector.tensor_scalar_add(
                            out=ptile[:, ic * IC_LEN:(ic + 1) * IC_LEN],
                            in0=pps[:],
                            scalar1=bias_t[p][:],
                        )
                    pt[(p, b)] = ptile

                # V natural layout + ones column: transpose V^T with the PE
                for h in range(H_PER_CORE):
                    v_t = vp_pool.tile([128, JT, 65], bf16, name=f"vp_{h}_{b}",
                                       tag=f"vp_{h}_{b}")
                    nc.vector.memset(v_t[:, :, 64:65], 1.0)
                    vp[(h, b)] = v_t
                for grp in range(2):  # noqa: B007
                    vps = ps_av.tile([128, 2, 512], f32, name=f"vps_{b}_{grp}",
                                     tag="av")
                    for j in range(8):
                        tt = grp * 8 + j
                        nc.tensor.matmul(
                            vps[:, j // 4, (j % 4) * 128:(j % 4) * 128 + 128],
                            lhsT=pt[("v", b)][:, tt * 128:(tt + 1) * 128],
                            rhs=identb[:],
                            start=True, stop=True,
                        )
                    for s in range(2):
                        blk = vps[:, s, :].rearrange("p (c m) -> p c m", m=128)
                        for h in range(H_PER_CORE):
                            nc.vector.tensor_copy(
                                out=vp[(h, b)][:, grp * 8 + s * 4:
                                               grp * 8 + s * 4 + 4, 0:64],
                                in_=blk[:, :, h * 64:h * 64 + 64],
                            )

            # --- attention -------------------------------------------------
            out_v = out.rearrange("(ic it p) b (h n) -> ic b h p it n",
                                  it=IT, p=128, h=H_PER_CORE)

            def attn_batch(b):
                for h in range(H_PER_CORE):
                    qv = pt[("q", b)]
                    kv = pt[("k", b)]
                    for ich in range(ICH):
                        # scores + exp for all 16 k-tiles of this i-chunk
                        esl = []
                        for jt in range(JT):
                            sps = ps_s.tile([128, IC_LEN], f32,
                                            name=f"sps_{b}_{h}_{ich}_{jt}", tag="s")
                            for s in range(2):
                                nc.tensor.matmul(
                                    sps[:, s * 512:(s + 1) * 512],
                                    lhsT=qv[h * 64:(h + 1) * 64,
                                            jt * 128:(jt + 1) * 128],
                                    rhs=kv[h * 64:(h + 1) * 64,
                                           ich * IC_LEN + s * 512:
                                           ich * IC_LEN + (s + 1) * 512],
                                    start=True, stop=True,
                                )
                            es = es_pool.tile([128, IC_LEN], bf16,
                                              name=f"es_{b}_{h}_{ich}_{jt}", tag="es")
                            nc.scalar.activation(out=es[:], in_=sps[:], func=AF.Exp,
                                                 scale=1.0 / 32.0)
                            esl.append(es)
                        # A @ [V, 1]: one 1-bank accumulator per i-tile so each
                        # PSUM zero region holds exactly one accumulation group
                        linv = sm_pool.tile([128, 8, 1], f32,
                                            name=f"linv_{b}_{h}_{ich}", tag="linv")
                        outf = of_pool.tile([128, IT, 64], f32,
                                            name=f"outf_{b}_{h}_{ich}", tag="of")
                        for it in range(IT):
                            av = ps_av.tile([128, 65], f32,
                                            name=f"av_{b}_{h}_{ich}_{it}", tag="av")
                            for jt in range(JT):
                                nc.tensor.matmul(
                                    av[:],
                                    lhsT=esl[jt][:, it * 128:(it + 1) * 128],
                                    rhs=vp[(h, b)][:, jt, :],
                                    start=(jt == 0), stop=(jt == JT - 1),
                                )
                            lv = linv[:, it:it + 1, :]
                            nc.vector.reciprocal(out=lv, in_=av[:, 64:65])
                            rep = bass.AP(tensor=lv.tensor, offset=lv.offset,
                                          ap=[lv.ap[0], [0, 64]])
                            nc.vector.tensor_mul(
                                out=outf[:, it, :],
                                in0=av[:, 0:64],
                                in1=rep,
                            )
                        nc.sync.dma_start(out=out_v[ich, b, h], in_=outf[:])

            for b in range(B):
                prep_batch(b)
            for b in range(B):
                attn_batch(b)
    nc.compile()  # bacc passes: regalloc, DCE, act-table loads, ...
    return nc


def _get_nc():
    if "nc" not in _CACHE:
        _CACHE["nc"] = _build()
    return _CACHE["nc"]


def run(inputs, trace=False, trace_kwargs=None):
    """Run on 8 NeuronCores. Returns (full_output, BassKernelResults)."""
    from concourse.bass_utils import run_bass_kernel_spmd

    nc = _get_nc()
    x = np.ascontiguousarray(np.asarray(inputs["x"], dtype=np.float32))
    in_maps = []
    for c in range(NCORES):
        sl = slice(c * GD, (c + 1) * GD)
        in_maps.append({
            "x": x,
            "wk": np.ascontiguousarray(np.asarray(inputs["Wk"], np.float32)[sl]),
            "wq": np.ascontiguousarray(np.asarray(inputs["Wq"], np.float32)[sl]),
            "wv": np.ascontiguousarray(np.asarray(inputs["Wv"], np.float32)[sl]),
            "bk": np.ascontiguousarray(np.asarray(inputs["bk"], np.float32)[sl]),
            "bq": np.ascontiguousarray(np.asarray(inputs["bq"], np.float32)[sl]),
            "bv": np.ascontiguousarray(np.asarray(inputs["bv"], np.float32)[sl]),
        })
    res = run_bass_kernel_spmd(nc, in_maps, core_ids=list(range(NCORES)),
                               trace=trace, **(trace_kwargs or {}))
    outs = [np.asarray(res.results[c]["out"]) for c in range(NCORES)]
    full = np.concatenate(outs, axis=2).astype(np.float32)
    return full, res


def kernel(x, mask, Wk, bk, Wq, bq, Wv, bv):
    """Full (unsharded) inputs -> full (T, B, H*N_V) float32 output.

    mask is all-True for this problem (spec fill: ones) and is ignored.
    """
    full, _ = run(dict(x=x, mask=mask, Wk=Wk, bk=bk, Wq=Wq, bq=bq, Wv=Wv, bv=bv))
    return full
